# revision 22
# baseline (speedup 1.0000x reference)
"""Masked dot-product attention on 8 TRN2 NeuronCores.

Math (per batch b):
    S = Q @ K^T / sqrt(64)                    [SQ, SK]
    S[:, k >= vl_b] = -1e6; A = softmax(S)    (masked cols -> weight 0)
    O = A @ V                                 [SQ, 64]

Device strategy (per core, SPMD -- identical instruction stream):
  * scores computed transposed: S_T[k, q] = sum_d K[k,d] Q[q,d] via
    matmul(lhsT=K^T tile [64,128], rhs=Q^T chunk [64,512]); two k-tiles
    (a "pair") run CONCURRENTLY on PE row groups h0/h64, writing one
    shared PSUM tile [128, 1024] = [tileA 512q | tileB 512q].
  * no max-subtraction: |S/8| <= ~6 so exp never overflows; masked keys
    have their V rows zeroed host-side so they contribute 0 to both
    numerator and denominator.
  * denominator via ones-column appended to V (host-side): row 64 of the
    accumulated O_aug^T is the softmax denominator; host divides.
  * exp is ONE instruction per pair ([128, 1024] PSUM -> f16 SBUF),
    alternating between ScalarE (native exp) and the otherwise-idle
    VectorE via a one-instruction Schraudolph bit-trick:
        f16_bits(e^s) ~= int16(round(s * 1024*log2(e)/8 + (15*1024 + C)))
    (tensor_scalar mult+add, int16 output through a bitcast view of the
    f16 pt tile; ~2% RMS on offloaded pairs, mean cancels in softmax).
  * PSUM: 3 pair-score tags (6 banks) + DOUBLE-BUFFERED acc (2 banks),
    so slot transitions never stall on the previous slot's drain.
  * PE order pinned in rounds of 3 pairs: [mm2s of the round 2 back,
    mm1s of this round] -- exp latency gets ~2 round-times of slack and
    stream-restart overhead is amortized over 3 pairs.
  * inputs stream in as a few ~1MB chunked DMAs into resident SBUF
    (each dma_start costs ~600ns serial sync-engine time + completion
    latency; chunks are chained at distance 2 so arrival matches
    consumption order); outputs are per-slot bf16.

Work scheduling: the host knows valid_lens at compile time, so each core
receives a host-packed list of (q-chunk "slot", k-tile "unit") work items
covering only k < vl. All cores run the same program shape; per-core
differences live entirely in the packed input data.
"""

import functools
import math

import numpy as np

B, SQ, SK, D = 16, 2048, 2048, 64
NCORES = 8
KT = 128          # k rows per unit (one matmul stationary tile)
QCH = 512         # q columns per slot
VA_W = D + 1      # V columns + ones column
VA_P = KT         # V_aug padded to 128 cols (full-width mm2: HAM + FWL)
PW = KT + 2 * VA_P  # merged pair row width: K^T pair cols + 2x padded V_aug

# Schraudolph exp constants (DVE path): bits = round(s*EXP_A + EXP_B)
EXP_A = 1024.0 * math.log2(math.e) / 8.0
EXP_B = 15360.0 - 57.6

DVE_FRAC = 0.38   # fraction of exp PAIRS offloaded to VectorE
ROUND = 3         # pairs per pinned PE round
LEAD = 2          # rounds the mm2 blocks trail their mm1s by

_last_results = None  # stashed BassKernelResults for test.py introspection


def _nkt(vl: int) -> int:
    return max(1, min(SK // KT, math.ceil(vl / KT)))


def _dve_pairs(npairs: int) -> set:
    """Pairs of a slot whose exp runs on VectorE (Schraudolph), spread
    evenly so the two exp engines stay co-busy within the slot."""
    k = round(npairs * DVE_FRAC)
    return {int((i + 0.5) * npairs / k) for i in range(k)} if k else set()


def _make_schedule(vl: np.ndarray, full: bool = False):
    """Assign the B*(SQ//QCH) (batch, q-chunk) slot-items to 8 cores,
    balanced by k-tile count. An item may be SPLIT across slots/cores
    (partial-k attention sums are additive; the host sums partial outputs
    before dividing).

    Returns (slot_sizes, assign): slot_sizes[s] is the compile-time unit
    count of slot s (identical on every core); assign[core][s] is
    (batch, qchunk, k_tile_start, n_real_ktiles) or None (pure padding)."""
    w = [SK // KT if full else _nkt(int(vl[b])) for b in range(B)]
    items = sorted(((b, h) for b in range(B) for h in range(SQ // QCH)),
                   key=lambda t: -w[t[0]])
    ngroups = len(items) // NCORES
    groups = [items[NCORES * s : NCORES * s + NCORES] for s in range(ngroups)]
    gmax = [max(w[b] for b, _ in g) for g in groups]
    gmin = [min(w[b] for b, _ in g) for g in groups]

    def evaluate(p):
        leftovers = []  # (len, batch, qchunk, k_start)
        for s, g in enumerate(groups):
            for b, h in g:
                if w[b] > p[s]:
                    leftovers.append((w[b] - p[s], b, h, p[s]))
        leftovers.sort(key=lambda t: -t[0])
        spares = []
        for i in range(0, len(leftovers), NCORES):
            spares.append(leftovers[i : i + NCORES])
        spare_sizes = [chunk[0][0] for chunk in spares]
        return sum(p) + sum(spare_sizes), spares, spare_sizes

    import itertools
    best = None
    ranges = [range(gmin[s], gmax[s] + 1) for s in range(ngroups)]
    ranges = [r if len(r) <= 3 else range(gmax[s] - 2, gmax[s] + 1)
              for s, r in zip(range(ngroups), ranges)]
    for p in itertools.product(*ranges):
        total, spares, spare_sizes = evaluate(list(p))
        cost = total + 0.7 * (len(p) + len(spares))
        if best is None or cost < best[0]:
            best = (cost, list(p), spares, spare_sizes)
    _, p, spares, spare_sizes = best

    slot_sizes = list(p) + spare_sizes
    assign = [[None] * len(slot_sizes) for _ in range(NCORES)]
    for s, g in enumerate(groups):
        for c, (b, h) in enumerate(g):
            assign[c][s] = (b, h, 0, min(w[b], p[s]))
    for k, chunk in enumerate(spares):
        for c, (ln, b, h, k_start) in enumerate(chunk):
            assign[c][ngroups + k] = (b, h, k_start, ln)
    # largest-first: the trailing slots are small, shortening the drain
    # tail after the last exp (slot transitions are cheap: acc ping-pong)
    order = sorted(range(len(slot_sizes)), key=lambda s: -slot_sizes[s])
    slot_sizes = [slot_sizes[s] for s in order]
    assign = [[a[s] for s in order] for a in assign]
    return tuple(slot_sizes), assign


def _input_layout(slot_sizes):
    """Column layout of the single packed input tensor [128, ncol] f16:
    per slot: qt block (QCH cols) then that slot's pair blocks (PW each,
    with internal K|V_A|V_B seams as chunk-cut candidates). Returns
    (ncol, qt_col[s], pair_col[global_pair], chunks)."""
    qt_col, pair_col, bounds = [], [], [0]
    col = 0
    for nu in slot_sizes:
        qt_col.append(col)
        col += QCH
        bounds.append(col)
        for _ in range((nu + 1) // 2):
            pair_col.append(col)
            col += PW
            bounds.extend([col - 2 * VA_P, col - VA_P, col])
    ncol = col
    chunks = []
    start = 0
    targets = iter([QCH + KT, 1536])
    target = next(targets)
    for bnd in bounds[1:]:
        if bnd - start >= target or bnd == ncol:
            chunks.append((start, bnd))
            start = bnd
            target = next(targets, 1792)
    return ncol, qt_col, pair_col, chunks


@functools.lru_cache(maxsize=4)
def _build_program(slot_sizes: tuple):
    """Build + schedule the SPMD Bass program for the given slot shape."""
    from collections import deque

    import concourse.bacc as bacc
    import concourse.mybir as mybir
    import concourse.tile as tile

    f32 = mybir.dt.float32
    f16 = mybir.dt.float16
    bf16 = mybir.dt.bfloat16
    i16 = mybir.dt.int16

    ncol, qt_col, pair_col, chunks = _input_layout(slot_sizes)
    n_slots = len(slot_sizes)

    nc = bacc.Bacc(
        "TRN2",
        target_bir_lowering=False,
        debug=False,
        enable_asserts=False,
        num_devices=NCORES,
    )
    inp = nc.dram_tensor("inp", [KT, ncol], f16, kind="ExternalInput")
    o = nc.dram_tensor("o", [n_slots, VA_W, QCH], bf16, kind="ExternalOutput")

    with tile.TileContext(nc) as tc:
        with (
            tc.tile_pool(name="inpool", bufs=1) as inpool,
            tc.tile_pool(name="ptpool", bufs=4) as ptpool,
            tc.tile_pool(name="opool", bufs=3) as opool,
            tc.tile_pool(name="scpool", bufs=1, space="PSUM") as scpool,
            tc.tile_pool(name="accpool", bufs=2, space="PSUM") as accpool,
        ):
            scale = 1.0 / math.sqrt(D)
            exp_f = mybir.ActivationFunctionType.Exp
            copy_f = mybir.ActivationFunctionType.Copy
            # dummy exp with no deps: pulls the ~2.7us ACT table load to
            # the very start of the kernel
            warm = inpool.tile([1, 8], f32, name="warm", tag="warm")
            nc.vector.memset(warm, 0.0)
            nc.scalar.activation(warm, warm, exp_f, scale=1.0)

            # input chunks, chained at distance 2 so arrival tracks
            # consumption order at near-full bandwidth
            ctiles = []
            dmas = []
            for ci, (c0, c1) in enumerate(chunks):
                ct = inpool.tile([KT, c1 - c0], f16, name=f"in{ci}",
                                 tag=f"in{ci}")
                dma = nc.sync.dma_start(out=ct, in_=inp[:, c0:c1])
                if ci >= 3:
                    tile.add_dep_helper(dma.ins, dmas[ci - 3].ins, True,
                                        "pace input stream")
                dmas.append(dma)
                ctiles.append((c0, c1, ct))

            def block(col, width):
                for c0, c1, ct in ctiles:
                    if col >= c0 and col + width <= c1:
                        return ct[:, col - c0 : col - c0 + width]
                raise AssertionError("block straddles chunk boundary")

            pe_tail = [None]

            def pe_pin(calls):
                for mcall in calls:
                    if pe_tail[0] is not None:
                        tile.add_dep_helper(mcall.ins, pe_tail[0].ins, False,
                                            "pe order")
                    pe_tail[0] = mcall

            # PE warm-up spin: ~4us of dummy back-to-back matmuls with no
            # data deps, so the HAM clock gate reaches K=8/8 BEFORE the
            # first data-dependent matmul (otherwise the DMA-paced early
            # phase keeps the PE at 1.2 GHz for ~10us). Writes the sc0
    # buffer; the first real sc0 mm1 chains behind via WAW.
            wz = inpool.tile([KT, QCH], f16, name="wz", tag="wz")
            nc.vector.memset(wz, 0.0)
            spin_sc = scpool.tile([KT, 2 * QCH], f32, name="spin", tag="sc0")
            pe_pin([nc.tensor.matmul(spin_sc[:, 0:QCH], lhsT=wz[0:64, 0:KT],
                                     rhs=wz[0:64, :], start=True, stop=True)
                    for _ in range(8)])

            round_mm1 = []       # mm1 calls accumulating for this round
            round_mm2 = []       # mm2 calls accumulating for this round
            rounds_pending = deque()  # mm2 blocks of the last LEAD rounds
            state = {"n": 0}

            def close_round():
                if not round_mm1:
                    return
                if len(rounds_pending) >= LEAD:
                    pe_pin(rounds_pending.popleft())
                pe_pin(list(round_mm1))
                rounds_pending.append(list(round_mm2))
                round_mm1.clear()
                round_mm2.clear()
                state["n"] = 0

            total_pairs = sum((nu + 1) // 2 for nu in slot_sizes)
            gp = 0   # global pair counter (sc/pt tag rotation)
            p_idx = 0
            for s, nu in enumerate(slot_sizes):
                dve = _dve_pairs((nu + 1) // 2)
                acc = accpool.tile([KT, QCH], f32)
                for jp in range((nu + 1) // 2):
                    pc = pair_col[p_idx]
                    p_idx += 1
                    lone = 2 * jp + 1 >= nu
                    sc = scpool.tile([KT, 2 * QCH], f32, name=f"sc_{gp}",
                                     tag=f"sc{gp % 3}")
                    pt = ptpool.tile([KT, 2 * QCH], f16, name=f"pt_{gp}",
                                     tag=f"pt{gp % 2}")
                    units = []
                    for half in (0, 1):
                        j = 2 * jp + half
                        real = not (lone and half == 1)
                        rows = slice(0, D) if half == 0 else slice(D, KT)
                        units.append((
                            j, real, rows,
                            block(pc, KT)[rows, :],               # K^T tile
                            block(pc + KT + half * VA_P, VA_P),   # V_aug
                        ))
                    qt_c = block(qt_col[s], QCH)
                    for j, real, rows, kt_t, va_t in units:
                        round_mm1.append(nc.tensor.matmul(
                            sc[:, (j % 2) * QCH : (j % 2 + 1) * QCH],
                            lhsT=kt_t,
                            rhs=qt_c[rows, :],
                            start=True,
                            stop=True,
                        ))
                    # ONE exp per pair over the whole [128, 1024] sc tile
                    if jp in dve:
                        nc.vector.tensor_scalar(
                            out=pt[:, :].bitcast(i16),
                            in0=sc[:, :],
                            scalar1=EXP_A,
                            scalar2=EXP_B,
                            op0=mybir.AluOpType.mult,
                            op1=mybir.AluOpType.add,
                        )
                    else:
                        nc.scalar.activation(pt, sc, exp_f, scale=scale)
                    for j, real, rows, kt_t, va_t in units:
                        if not real:
                            continue
                        round_mm2.append(nc.tensor.matmul(
                            acc[:, :],
                            lhsT=va_t,
                            rhs=pt[:, (j % 2) * QCH : (j % 2 + 1) * QCH],
                            start=(j == 0),
                            stop=(j == nu - 1),
                        ))
                    gp += 1
                    state["n"] += 1
                    if state["n"] >= ROUND:
                        close_round()
                # drain acc -> SBUF bf16 (acc double-buffered: the next
                # slot's mm2s never wait on this); ONE output DMA per slot
                o_sb = opool.tile([VA_W, QCH], bf16)
                last = s == n_slots - 1
                if last:
                    nc.vector.tensor_copy(o_sb[:, 0:256], acc[0:VA_W, 0:256])
                    nc.scalar.activation(o_sb[:, 256:QCH],
                                         acc[0:VA_W, 256:QCH], copy_f)
                else:
                    nc.vector.tensor_copy(o_sb, acc[0:VA_W, :])
                nc.sync.dma_start(out=o[s], in_=o_sb)
            close_round()
            while rounds_pending:
                pe_pin(rounds_pending.popleft())
    nc.compile()
    return nc


def _pack_inputs(queries, keys, values, vl, slot_sizes, assign):
    """Build each core's packed device input per its schedule (mirrors the
    device program's layout exactly)."""
    ncol, qt_col, pair_col, _ = _input_layout(slot_sizes)
    qT = np.ascontiguousarray(queries.transpose(0, 2, 1).astype(np.float16))
    kT = keys.astype(np.float16)  # [B, SK, D] row-major, sliced per k-tile
    in_maps = []
    for c in range(NCORES):
        inp = np.zeros((KT, ncol), np.float16)
        p_idx = 0
        for s, nu in enumerate(slot_sizes):
            if assign[c][s] is None:
                p_idx += (nu + 1) // 2
                continue  # pure-padding slot: all-zero inputs contribute 0
            b, h, ks, w = assign[c][s]
            qc = qt_col[s]
            inp[:D, qc : qc + QCH] = qT[b, :, h * QCH : (h + 1) * QCH]
            inp[D:KT, qc : qc + QCH] = inp[:D, qc : qc + QCH]
            nvalid = int(vl[b])
            for jp in range((nu + 1) // 2):
                pc = pair_col[p_idx]
                for half in (0, 1):
                    # a lone unit's B half is a dummy mm1 partner (device
                    # skips its mm2): real K data keeps array activity up
                    j = min(2 * jp + half, nu - 1)
                    t = ks + min(j, w - 1)  # padding units replay a k-tile
                    rows = slice(0, D) if half == 0 else slice(D, KT)
                    inp[rows, pc : pc + KT] = kT[b, t * KT : (t + 1) * KT, :].T
                    if j < w and not (half == 1 and 2 * jp + 1 >= nu):
                        k0 = t * KT
                        nv = min(max(nvalid - k0, 0), KT)
                        col0 = pc + KT + half * VA_P
                        inp[:nv, col0 : col0 + D] = values[b, k0 : k0 + nv, :]
                        inp[:nv, col0 + D] = 1.0
                    # padding units leave V_aug zero -> contribute nothing
                p_idx += 1
        in_maps.append({"inp": inp})
    return in_maps


def kernel(queries, keys, values, valid_lens, _full=False, _trace=False):
    global _last_results
    from concourse.bass_utils import run_bass_kernel_spmd

    queries = np.ascontiguousarray(np.asarray(queries, dtype=np.float32))
    keys = np.ascontiguousarray(np.asarray(keys, dtype=np.float32))
    values = np.ascontiguousarray(np.asarray(values, dtype=np.float32))
    vl = np.asarray(valid_lens).astype(np.int64).reshape(B)

    slot_sizes, assign = _make_schedule(vl, full=_full)
    nc = _build_program(slot_sizes)
    in_maps = _pack_inputs(queries, keys, values, vl, slot_sizes, assign)

    kwargs = {"trace": True} if _trace else {}
    res = run_bass_kernel_spmd(nc, in_maps, core_ids=list(range(NCORES)), **kwargs)
    _last_results = res

    # Sum partial (numerator, denominator) contributions per (batch,
    # q-chunk), then divide once -- exact for split items.
    agg = np.zeros((B, SQ // QCH, VA_W, QCH), np.float64)
    for c in range(NCORES):
        oc = np.asarray(res.results[c]["o"], dtype=np.float32)
        for s in range(len(slot_sizes)):
            if assign[c][s] is None:
                continue
            b, h, _, _ = assign[c][s]
            agg[b, h] += oc[s]
    out = np.empty((B, SQ, D), np.float32)
    for b in range(B):
        for h in range(SQ // QCH):
            num = agg[b, h, :D, :]
            den = agg[b, h, D, :]
            out[b, h * QCH : (h + 1) * QCH, :] = (num / den).T.astype(np.float32)
    return out


# revision 23
# speedup vs baseline: 1.1703x; 1.1703x over previous
"""Masked dot-product attention on 8 TRN2 NeuronCores.

Math (per batch b):
    S = Q @ K^T / sqrt(64)                    [SQ, SK]
    S[:, k >= vl_b] = -1e6; A = softmax(S)    (masked cols -> weight 0)
    O = A @ V                                 [SQ, 64]

Device strategy (per core, SPMD -- identical instruction stream):
  * scores computed transposed: S_T[k, q] = sum_d K[k,d] Q[q,d] via
    matmul(lhsT=K^T tile [64,128], rhs=Q^T chunk [64,512]); two k-tiles
    (a "pair") run CONCURRENTLY on PE row groups h0/h64, writing one
    shared PSUM tile [128, 1024] = [tileA 512q | tileB 512q].
  * no max-subtraction: |S/8| <= ~6 so exp never overflows; masked keys
    have their V rows zeroed host-side so they contribute 0 to both
    numerator and denominator.
  * denominator via ones-column appended to V (host-side): row 64 of the
    accumulated O_aug^T is the softmax denominator; host divides.
  * exp is ONE instruction per pair ([128, 1024] PSUM -> f16 SBUF),
    alternating between ScalarE (native exp) and the otherwise-idle
    VectorE via a one-instruction Schraudolph bit-trick:
        f16_bits(e^s) ~= int16(round(s * 1024*log2(e)/8 + (15*1024 + C)))
    (tensor_scalar mult+add, int16 output through a bitcast view of the
    f16 pt tile; ~2% RMS on offloaded pairs, mean cancels in softmax).
  * PSUM: 3 pair-score tags (6 banks) + DOUBLE-BUFFERED acc (2 banks),
    so slot transitions never stall on the previous slot's drain.
  * PE order pinned in rounds of 3 pairs: [mm2s of the round 2 back,
    mm1s of this round] -- exp latency gets ~2 round-times of slack and
    stream-restart overhead is amortized over 3 pairs.
  * inputs stream in as a few ~1MB chunked DMAs into resident SBUF
    (each dma_start costs ~600ns serial sync-engine time + completion
    latency; chunks are chained at distance 2 so arrival matches
    consumption order); outputs are per-slot bf16.

Work scheduling: the host knows valid_lens at compile time, so each core
receives a host-packed list of (q-chunk "slot", k-tile "unit") work items
covering only k < vl. All cores run the same program shape; per-core
differences live entirely in the packed input data.
"""

import functools
import math

import numpy as np

B, SQ, SK, D = 16, 2048, 2048, 64
NCORES = 8
KT = 128          # k rows per unit (one matmul stationary tile)
QCH = 512         # q columns per slot
VA_W = D + 1      # V columns + ones column
VA_P = KT         # V_aug padded to 128 cols (full-width mm2: HAM + FWL)
PW = KT + 2 * VA_P  # merged pair row width: K^T pair cols + 2x padded V_aug

# Schraudolph exp constants (DVE path): bits = round(s*EXP_A + EXP_B)
EXP_A = 1024.0 * math.log2(math.e) / 8.0
EXP_B = 15360.0 - 57.6

DVE_FRAC = 0.38   # fraction of exp PAIRS offloaded to VectorE
ROUND = 3         # pairs per pinned PE round
LEAD = 2          # rounds the mm2 blocks trail their mm1s by

_last_results = None  # stashed BassKernelResults for test.py introspection


def _nkt(vl: int) -> int:
    return max(1, min(SK // KT, math.ceil(vl / KT)))


def _dve_pairs(npairs: int) -> set:
    """Pairs of a slot whose exp runs on VectorE (Schraudolph), spread
    evenly so the two exp engines stay co-busy within the slot."""
    k = round(npairs * DVE_FRAC)
    return {int((i + 0.5) * npairs / k) for i in range(k)} if k else set()


def _make_schedule(vl: np.ndarray, full: bool = False):
    """Assign the B*(SQ//QCH) (batch, q-chunk) slot-items to 8 cores,
    balanced by k-tile count. An item may be SPLIT across slots/cores
    (partial-k attention sums are additive; the host sums partial outputs
    before dividing).

    Returns (slot_sizes, assign): slot_sizes[s] is the compile-time unit
    count of slot s (identical on every core); assign[core][s] is
    (batch, qchunk, k_tile_start, n_real_ktiles) or None (pure padding)."""
    w = [SK // KT if full else _nkt(int(vl[b])) for b in range(B)]
    items = sorted(((b, h) for b in range(B) for h in range(SQ // QCH)),
                   key=lambda t: -w[t[0]])
    ngroups = len(items) // NCORES
    groups = [items[NCORES * s : NCORES * s + NCORES] for s in range(ngroups)]
    gmax = [max(w[b] for b, _ in g) for g in groups]
    gmin = [min(w[b] for b, _ in g) for g in groups]

    def evaluate(p):
        leftovers = []  # (len, batch, qchunk, k_start)
        for s, g in enumerate(groups):
            for b, h in g:
                if w[b] > p[s]:
                    leftovers.append((w[b] - p[s], b, h, p[s]))
        leftovers.sort(key=lambda t: -t[0])
        spares = []
        for i in range(0, len(leftovers), NCORES):
            spares.append(leftovers[i : i + NCORES])
        spare_sizes = [chunk[0][0] for chunk in spares]
        return sum(p) + sum(spare_sizes), spares, spare_sizes

    import itertools
    best = None
    ranges = [range(gmin[s], gmax[s] + 1) for s in range(ngroups)]
    ranges = [r if len(r) <= 3 else range(gmax[s] - 2, gmax[s] + 1)
              for s, r in zip(range(ngroups), ranges)]
    for p in itertools.product(*ranges):
        total, spares, spare_sizes = evaluate(list(p))
        cost = total + 0.7 * (len(p) + len(spares))
        if best is None or cost < best[0]:
            best = (cost, list(p), spares, spare_sizes)
    _, p, spares, spare_sizes = best

    slot_sizes = list(p) + spare_sizes
    assign = [[None] * len(slot_sizes) for _ in range(NCORES)]
    for s, g in enumerate(groups):
        for c, (b, h) in enumerate(g):
            assign[c][s] = (b, h, 0, min(w[b], p[s]))
    for k, chunk in enumerate(spares):
        for c, (ln, b, h, k_start) in enumerate(chunk):
            assign[c][ngroups + k] = (b, h, k_start, ln)
    # largest-first: the trailing slots are small, shortening the drain
    # tail after the last exp (slot transitions are cheap: acc ping-pong)
    order = sorted(range(len(slot_sizes)), key=lambda s: -slot_sizes[s])
    slot_sizes = [slot_sizes[s] for s in order]
    assign = [[a[s] for s in order] for a in assign]
    return tuple(slot_sizes), assign


def _input_layout(slot_sizes):
    """Column layout of the single packed input tensor [128, ncol] f16:
    per slot: qt block (QCH cols) then that slot's pair blocks (PW each,
    with internal K|V_A|V_B seams as chunk-cut candidates). Returns
    (ncol, qt_col[s], pair_col[global_pair], chunks)."""
    qt_col, pair_col, bounds = [], [], [0]
    col = 0
    for nu in slot_sizes:
        qt_col.append(col)
        col += QCH
        bounds.append(col)
        for _ in range((nu + 1) // 2):
            pair_col.append(col)
            col += PW
            bounds.extend([col - 2 * VA_P, col - VA_P, col])
    ncol = col
    chunks = []
    start = 0
    targets = iter([QCH + KT, 2048, 3584])
    target = next(targets)
    for bnd in bounds[1:]:
        if bnd - start >= target or bnd == ncol:
            chunks.append((start, bnd))
            start = bnd
            target = next(targets, 3584)
    return ncol, qt_col, pair_col, chunks


@functools.lru_cache(maxsize=4)
def _build_program(slot_sizes: tuple):
    """Build + schedule the SPMD Bass program for the given slot shape."""
    from collections import deque

    import concourse.bacc as bacc
    import concourse.mybir as mybir
    import concourse.tile as tile

    f32 = mybir.dt.float32
    f16 = mybir.dt.float16
    bf16 = mybir.dt.bfloat16
    i16 = mybir.dt.int16

    ncol, qt_col, pair_col, chunks = _input_layout(slot_sizes)
    n_slots = len(slot_sizes)

    nc = bacc.Bacc(
        "TRN2",
        target_bir_lowering=False,
        debug=False,
        enable_asserts=False,
        num_devices=NCORES,
    )
    inp = nc.dram_tensor("inp", [KT, ncol], f16, kind="ExternalInput")
    o = nc.dram_tensor("o", [n_slots, VA_W, QCH], bf16, kind="ExternalOutput")

    with tile.TileContext(nc) as tc:
        with (
            tc.tile_pool(name="inpool", bufs=1) as inpool,
            tc.tile_pool(name="ptpool", bufs=4) as ptpool,
            tc.tile_pool(name="opool", bufs=3) as opool,
            tc.tile_pool(name="scpool", bufs=1, space="PSUM") as scpool,
            tc.tile_pool(name="accpool", bufs=2, space="PSUM") as accpool,
        ):
            scale = 1.0 / math.sqrt(D)
            exp_f = mybir.ActivationFunctionType.Exp
            copy_f = mybir.ActivationFunctionType.Copy
            # dummy exp with no deps: pulls the ~2.7us ACT table load to
            # the very start of the kernel
            warm = inpool.tile([1, 8], f32, name="warm", tag="warm")
            nc.vector.memset(warm, 0.0)
            nc.scalar.activation(warm, warm, exp_f, scale=1.0)

            # input chunks, chained at distance 2 so arrival tracks
            # consumption order at near-full bandwidth
            ctiles = []
            dmas = []
            for ci, (c0, c1) in enumerate(chunks):
                ct = inpool.tile([KT, c1 - c0], f16, name=f"in{ci}",
                                 tag=f"in{ci}")
                dma = nc.sync.dma_start(out=ct, in_=inp[:, c0:c1])
                if ci >= 2:
                    tile.add_dep_helper(dma.ins, dmas[ci - 2].ins, True,
                                        "pace input stream")
                dmas.append(dma)
                ctiles.append((c0, c1, ct))

            def block(col, width):
                for c0, c1, ct in ctiles:
                    if col >= c0 and col + width <= c1:
                        return ct[:, col - c0 : col - c0 + width]
                raise AssertionError("block straddles chunk boundary")

            pe_tail = [None]

            def pe_pin(calls):
                for mcall in calls:
                    if pe_tail[0] is not None:
                        tile.add_dep_helper(mcall.ins, pe_tail[0].ins, False,
                                            "pe order")
                    pe_tail[0] = mcall

            # PE warm-up spin: ~4us of dummy back-to-back matmuls with no
            # data deps, so the HAM clock gate reaches K=8/8 BEFORE the
            # first data-dependent matmul (otherwise the DMA-paced early
            # phase keeps the PE at 1.2 GHz for ~10us). Writes the sc0
    # buffer; the first real sc0 mm1 chains behind via WAW.
            wz = inpool.tile([KT, QCH], f16, name="wz", tag="wz")
            nc.vector.memset(wz, 0.0)
            spin_sc = scpool.tile([KT, 2 * QCH], f32, name="spin", tag="sc0")
            pe_pin([nc.tensor.matmul(spin_sc[:, 0:QCH], lhsT=wz[0:64, 0:KT],
                                     rhs=wz[0:64, :], start=True, stop=True)
                    for _ in range(10)])

            round_mm1 = []       # mm1 calls accumulating for this round
            round_mm2 = []       # mm2 calls accumulating for this round
            rounds_pending = deque()  # mm2 blocks of the last LEAD rounds
            state = {"n": 0}

            def close_round():
                if not round_mm1:
                    return
                if len(rounds_pending) >= LEAD:
                    pe_pin(rounds_pending.popleft())
                pe_pin(list(round_mm1))
                rounds_pending.append(list(round_mm2))
                round_mm1.clear()
                round_mm2.clear()
                state["n"] = 0

            gp = 0   # global pair counter (sc/pt tag rotation)
            p_idx = 0
            for s, nu in enumerate(slot_sizes):
                dve = _dve_pairs((nu + 1) // 2)
                acc = accpool.tile([KT, QCH], f32)
                for jp in range((nu + 1) // 2):
                    pc = pair_col[p_idx]
                    p_idx += 1
                    lone = 2 * jp + 1 >= nu
                    sc = scpool.tile([KT, 2 * QCH], f32, name=f"sc_{gp}",
                                     tag=f"sc{gp % 3}")
                    pt = ptpool.tile([KT, 2 * QCH], f16, name=f"pt_{gp}",
                                     tag=f"pt{gp % 2}")
                    units = []
                    for half in (0, 1):
                        j = 2 * jp + half
                        real = not (lone and half == 1)
                        rows = slice(0, D) if half == 0 else slice(D, KT)
                        units.append((
                            j, real, rows,
                            block(pc, KT)[rows, :],               # K^T tile
                            block(pc + KT + half * VA_P, VA_P),   # V_aug
                        ))
                    qt_c = block(qt_col[s], QCH)
                    for j, real, rows, kt_t, va_t in units:
                        round_mm1.append(nc.tensor.matmul(
                            sc[:, (j % 2) * QCH : (j % 2 + 1) * QCH],
                            lhsT=kt_t,
                            rhs=qt_c[rows, :],
                            start=True,
                            stop=True,
                        ))
                    # ONE exp per pair over the whole [128, 1024] sc tile
                    if jp in dve:
                        nc.vector.tensor_scalar(
                            out=pt[:, :].bitcast(i16),
                            in0=sc[:, :],
                            scalar1=EXP_A,
                            scalar2=EXP_B,
                            op0=mybir.AluOpType.mult,
                            op1=mybir.AluOpType.add,
                        )
                    else:
                        nc.scalar.activation(pt, sc, exp_f, scale=scale)
                    for j, real, rows, kt_t, va_t in units:
                        if not real:
                            continue
                        round_mm2.append(nc.tensor.matmul(
                            acc[:, :],
                            lhsT=va_t,
                            rhs=pt[:, (j % 2) * QCH : (j % 2 + 1) * QCH],
                            start=(j == 0),
                            stop=(j == nu - 1),
                        ))
                    gp += 1
                    state["n"] += 1
                    if state["n"] >= ROUND:
                        close_round()
                # drain acc -> SBUF bf16 (acc double-buffered: the next
                # slot's mm2s never wait on this); ONE output DMA per slot
                o_sb = opool.tile([VA_W, QCH], bf16)
                last = s == n_slots - 1
                if last:
                    nc.vector.tensor_copy(o_sb[:, 0:256], acc[0:VA_W, 0:256])
                    nc.scalar.activation(o_sb[:, 256:QCH],
                                         acc[0:VA_W, 256:QCH], copy_f)
                else:
                    nc.vector.tensor_copy(o_sb, acc[0:VA_W, :])
                nc.sync.dma_start(out=o[s], in_=o_sb)
            close_round()
            while rounds_pending:
                pe_pin(rounds_pending.popleft())
    nc.compile()
    return nc


def _pack_inputs(queries, keys, values, vl, slot_sizes, assign):
    """Build each core's packed device input per its schedule (mirrors the
    device program's layout exactly)."""
    ncol, qt_col, pair_col, _ = _input_layout(slot_sizes)
    qT = np.ascontiguousarray(queries.transpose(0, 2, 1).astype(np.float16))
    kT = keys.astype(np.float16)  # [B, SK, D] row-major, sliced per k-tile
    in_maps = []
    for c in range(NCORES):
        inp = np.zeros((KT, ncol), np.float16)
        p_idx = 0
        for s, nu in enumerate(slot_sizes):
            if assign[c][s] is None:
                p_idx += (nu + 1) // 2
                continue  # pure-padding slot: all-zero inputs contribute 0
            b, h, ks, w = assign[c][s]
            qc = qt_col[s]
            inp[:D, qc : qc + QCH] = qT[b, :, h * QCH : (h + 1) * QCH]
            inp[D:KT, qc : qc + QCH] = inp[:D, qc : qc + QCH]
            nvalid = int(vl[b])
            for jp in range((nu + 1) // 2):
                pc = pair_col[p_idx]
                for half in (0, 1):
                    # a lone unit's B half is a dummy mm1 partner (device
                    # skips its mm2): real K data keeps array activity up
                    j = min(2 * jp + half, nu - 1)
                    t = ks + min(j, w - 1)  # padding units replay a k-tile
                    rows = slice(0, D) if half == 0 else slice(D, KT)
                    inp[rows, pc : pc + KT] = kT[b, t * KT : (t + 1) * KT, :].T
                    if j < w and not (half == 1 and 2 * jp + 1 >= nu):
                        k0 = t * KT
                        nv = min(max(nvalid - k0, 0), KT)
                        col0 = pc + KT + half * VA_P
                        inp[:nv, col0 : col0 + D] = values[b, k0 : k0 + nv, :]
                        inp[:nv, col0 + D] = 1.0
                    # padding units leave V_aug zero -> contribute nothing
                p_idx += 1
        in_maps.append({"inp": inp})
    return in_maps


def kernel(queries, keys, values, valid_lens, _full=False, _trace=False):
    global _last_results
    from concourse.bass_utils import run_bass_kernel_spmd

    queries = np.ascontiguousarray(np.asarray(queries, dtype=np.float32))
    keys = np.ascontiguousarray(np.asarray(keys, dtype=np.float32))
    values = np.ascontiguousarray(np.asarray(values, dtype=np.float32))
    vl = np.asarray(valid_lens).astype(np.int64).reshape(B)

    slot_sizes, assign = _make_schedule(vl, full=_full)
    nc = _build_program(slot_sizes)
    in_maps = _pack_inputs(queries, keys, values, vl, slot_sizes, assign)

    kwargs = {"trace": True} if _trace else {}
    res = run_bass_kernel_spmd(nc, in_maps, core_ids=list(range(NCORES)), **kwargs)
    _last_results = res

    # Sum partial (numerator, denominator) contributions per (batch,
    # q-chunk), then divide once -- exact for split items.
    agg = np.zeros((B, SQ // QCH, VA_W, QCH), np.float64)
    for c in range(NCORES):
        oc = np.asarray(res.results[c]["o"], dtype=np.float32)
        for s in range(len(slot_sizes)):
            if assign[c][s] is None:
                continue
            b, h, _, _ = assign[c][s]
            agg[b, h] += oc[s]
    out = np.empty((B, SQ, D), np.float32)
    for b in range(B):
        for h in range(SQ // QCH):
            num = agg[b, h, :D, :]
            den = agg[b, h, D, :]
            out[b, h * QCH : (h + 1) * QCH, :] = (num / den).T.astype(np.float32)
    return out
